# revision 7
# baseline (speedup 1.0000x reference)
"""Causal self-attention (B=2, S=2048, D=1024, H=16, hd=64) on 8 TRN2 NeuronCores.

Sharding: batch x head-group. Core c handles batch c//4 and heads
4*(c%4) .. 4*(c%4)+3. Each core computes its 4 heads' attention plus the
partial output projection; the host sums the 4 partial projections per batch.

v2 (vs the 239us baseline):
  - inputs host-pretiled to [128, K*cols] so each tensor loads with one
    contiguous-per-partition DMA; DMAs spread over 4 engine queues so the
    ~1us SWDGE descriptor-gen per dma_start parallelizes (compute starts
    ~3us instead of ~28us).
  - gpsimd ISA library preloaded with a dummy partition_broadcast at t=0
    (the lazy lib load cost ~7us on the first chunk's denominator chain).
  - scores / exp / mask exploit causality inside the diagonal 512-chunk:
    cols < 128*r of a diagonal key-tile are skipped (matmul + exp trimmed,
    probs zero-memset), the 0/1 mask multiply shrinks to the [128,128]
    triangle. Exp for the head pair is one [128, 2, cols] instruction.
  - denominator chain per (hp,e): copy PSUM->SBUF f16 (frees the PSUM
    accumulator ~0.6us after the last AV), reciprocal of the sum row,
    gpsimd partition_broadcast, one f16 multiply. avps needs only 2 banks.
  - output projection of chunk i is emitted inside chunk i+1's score loop
    (PE filler while Act runs exp), chunks processed in order 0,3,2,1 so
    the serial tail is the smallest chunk; y stored f16, one DMA per chunk.
"""

import sys

try:
    import concourse.bass  # noqa: F401
except ImportError:
    sys.path.insert(0, "/opt/trn_rl_repo")

import numpy as np
import concourse.bacc as bacc
import concourse.mybir as mybir
from concourse.tile import TileContext
from concourse.bass_utils import run_bass_kernel_spmd

F32 = mybir.dt.float32
F16 = mybir.dt.float16

B, S, D = 2, 2048, 1024
H, HD = 16, 64
HEADS_PER_CORE = 4
N_CORES = 8
ROPE_BASE = 10000.0
SCALE = HD ** -0.5

KT = D // 128          # 8  contraction tiles for the QKV projection
ST = S // 128          # 16 sequence tiles of 128
NC_CH = S // 512       # 4  sequence chunks of 512
WF = 3 * HEADS_PER_CORE * HD   # 768 projection features per core
VOFF = 2 * HEADS_PER_CORE * HD # 512 column offset of the v block in w

CHUNK_ORDER = [0, 3, 2, 1]


def _build_program():
    nc = bacc.Bacc("TRN2", target_bir_lowering=False, debug=False,
                   num_devices=N_CORES)

    xT = nc.dram_tensor("xT", [128, KT * S], F16, kind="ExternalInput")
    w = nc.dram_tensor("w", [128, KT * WF], F16, kind="ExternalInput")
    wo = nc.dram_tensor("wo", [128, 2 * D], F16, kind="ExternalInput")
    cosT = nc.dram_tensor("cosT", [128, S], F16, kind="ExternalInput")
    sinT = nc.dram_tensor("sinT", [128, S], F16, kind="ExternalInput")
    rmatT = nc.dram_tensor("rmatT", [128, 128], F16, kind="ExternalInput")
    mask2 = nc.dram_tensor("mask2", [128, 256], F16, kind="ExternalInput")
    y = nc.dram_tensor("y", [S, D], F16, kind="ExternalOutput")

    with TileContext(nc) as tc:
        with (
            tc.tile_pool(name="const", bufs=1) as constp,
            tc.tile_pool(name="acts", bufs=1) as actsp,
        ):
            w_sb = constp.tile([128, KT * WF], F16)
            wo_sb = constp.tile([128, 2 * D], F16)
            cos_sb = constp.tile([128, S], F16)
            sin_sb = constp.tile([128, S], F16)
            rmat_sb = constp.tile([128, 128], F16)
            mask_sb = constp.tile([128, 256], F16)
            warm_sb = constp.tile([128, 8], F16)

            # gpsimd ISA library preload: a dummy broadcast at t=0 so the
            # ~7us lazy lib load overlaps the input DMAs.
            nc.vector.memset(warm_sb[0:1, :], 1.0)
            nc.gpsimd.partition_broadcast(warm_sb[64:128, :], warm_sb[0:1, :])

            # input DMAs: only SP/Act (HWDGE) and gpsimd (SWDGE) can issue.
            # x tiles on sync, w tiles + small constants on scalar, bulky
            # late-needed constants on gpsimd (queued behind the lib load).

            # activations produced by the QKV phase, consumed by attention
            qT_sb = actsp.tile([128, 2 * S], F16)   # head pairs 0|1
            kT_sb = actsp.tile([128, 2 * S], F16)
            v_sb = actsp.tile([128, ST * 260], F16) # 16 seq tiles x 4x65
            outT_sb = actsp.tile([128, 2 * S], F16)

            # ones columns of the v blocks (col 64 of each 65-block)
            ones_cols = v_sb[:, 0:ST * 260].rearrange(
                "p (b c) -> p b c", c=65)[:, :, 64:65]
            nc.vector.memset(ones_cols, 1.0)

            # ---------------- QKV projection + RoPE ----------------
            with (
                tc.tile_pool(name="xt", bufs=1) as xtp,
                tc.tile_pool(name="qkps", bufs=4, space="PSUM") as qkps,
                tc.tile_pool(name="rotps", bufs=2, space="PSUM") as rotps,
                tc.tile_pool(name="vps", bufs=2, space="PSUM") as vps,
                tc.tile_pool(name="qpre", bufs=2) as qprep,
                tc.tile_pool(name="ropet", bufs=2) as ropetp,
            ):
                xT_sb = xtp.tile([128, KT * S], F16)
                # k ascending so the mt=0 accumulation paces with arrivals
                for k in range(KT):
                    nc.sync.dma_start(
                        xT_sb[:, k * S:(k + 1) * S], xT[:, k * S:(k + 1) * S])
                    nc.scalar.dma_start(
                        w_sb[:, k * WF:(k + 1) * WF], w[:, k * WF:(k + 1) * WF])
                nc.scalar.dma_start(rmat_sb[:], rmatT[:])
                nc.scalar.dma_start(mask_sb[:], mask2[:])
                nc.gpsimd.dma_start(cos_sb[:], cosT[:])
                nc.gpsimd.dma_start(sin_sb[:], sinT[:])
                nc.gpsimd.dma_start(wo_sb[:], wo[:])

                # q/k head-pair tiles: mt 0,1 -> q pairs; 2,3 -> k pairs
                for mt in range(4):
                    dest = qT_sb if mt < 2 else kT_sb
                    doff = (mt % 2) * S
                    pts = [qkps.tile([128, 512], F32, name=f"qkpsum{_n}",
                                     tag="qkpsum") for _n in range(NC_CH)]
                    for k in range(KT):
                        lhsT = w_sb[:, k * WF + mt * 128: k * WF + (mt + 1) * 128]
                        for n in range(NC_CH):
                            nc.tensor.matmul(
                                pts[n][:],
                                lhsT,
                                xT_sb[:, k * S + n * 512: k * S + (n + 1) * 512],
                                start=(k == 0), stop=(k == KT - 1))
                    for n in range(NC_CH):
                        qpre = qprep.tile([128, 512], F16)
                        nc.scalar.copy(qpre[:], pts[n][:])
                        rot = rotps.tile([128, 512], F32)
                        nc.tensor.matmul(rot[:], rmat_sb[:], qpre[:],
                                         start=True, stop=True)
                        t1 = ropetp.tile([128, 512], F16, tag="t1")
                        t2 = ropetp.tile([128, 512], F16, tag="t2")
                        nc.vector.tensor_mul(
                            t1[:], qpre[:], cos_sb[:, n * 512:(n + 1) * 512])
                        nc.vector.tensor_mul(
                            t2[:], rot[:], sin_sb[:, n * 512:(n + 1) * 512])
                        nc.vector.tensor_add(
                            dest[:, doff + n * 512: doff + (n + 1) * 512],
                            t1[:], t2[:])

                # v in [seq, head-block] layout
                for st in range(ST):
                    pv = vps.tile([128, 256], F32)
                    for k in range(KT):
                        nc.tensor.matmul(
                            pv[:],
                            xT_sb[:, k * S + st * 128: k * S + (st + 1) * 128],
                            w_sb[:, k * WF + VOFF: k * WF + WF],
                            start=(k == 0), stop=(k == KT - 1))
                    vdst = v_sb[:, st * 260:(st + 1) * 260].rearrange(
                        "p (h c) -> p h c", c=65)[:, :, 0:64]
                    nc.vector.tensor_copy(
                        vdst, pv[:].rearrange("p (h c) -> p h c", c=64))

            # ---------------- attention + output projection ----------------
            with (
                tc.tile_pool(name="scps", bufs=2, space="PSUM") as scps,
                tc.tile_pool(name="avps", bufs=1, space="PSUM") as avps,
                tc.tile_pool(name="yps", bufs=2, space="PSUM") as yps,
                tc.tile_pool(name="probs", bufs=4) as probsp,
                tc.tile_pool(name="outu", bufs=2) as outup,
                tc.tile_pool(name="rrp", bufs=2) as rrp,
                tc.tile_pool(name="binv", bufs=2) as binvp,
                tc.tile_pool(name="ysb", bufs=2) as ysbp,
            ):
                mask3 = mask_sb[:, 0:256].rearrange("p (b c) -> p b c", b=2)

                # deferred output-projection units; each unit is one
                # (st, nn) pair: 2 accumulating matmuls + a PSUM->SBUF f16
                # copy into the chunk's staging tile, DMA after the last.
                pending = []   # list of closures for the previous chunk

                def make_units(pc):
                    ycb = {}

                    def unit(u, pc=pc, ycb=ycb):
                        if u == 0:
                            ycb["t"] = ysbp.tile([128, 4096], F16, name="ycb",
                                                 tag="ycb")
                        sti, nn = u // 2, u % 2
                        st = pc * 4 + sti
                        py = yps.tile([128, 512], F32, name="py", tag="py")
                        for hp2 in range(2):
                            nc.tensor.matmul(
                                py[:],
                                outT_sb[:, hp2 * S + st * 128: hp2 * S + (st + 1) * 128],
                                wo_sb[:, hp2 * D + nn * 512: hp2 * D + (nn + 1) * 512],
                                start=(hp2 == 0), stop=(hp2 == 1))
                        nc.vector.tensor_copy(
                            ycb["t"][:, sti * 1024 + nn * 512: sti * 1024 + (nn + 1) * 512],
                            py[:])
                        if u == 7:
                            dst = y[pc * 512:(pc + 1) * 512, :].rearrange(
                                "(s p) d -> p s d", p=128)
                            nc.sync.dma_start(
                                dst, ycb["t"][:].rearrange("p (s d) -> p s d", s=4))
                    return [lambda u=u: unit(u) for u in range(8)]

                for ic in CHUNK_ORDER:
                    jmax = 4 * ic + 4
                    qoffc = ic * 512
                    for hp in range(2):
                        qoff = hp * S
                        pav = [avps.tile([128, 512], F32, name=f"av{e}",
                                         tag=f"av{e}") for e in range(2)]
                        prev = None   # (jt, probs tile)
                        for jt in range(jmax):
                            r = jt - 4 * ic
                            c0 = 128 * r if r > 0 else 0
                            ps = scps.tile([128, 1024], F32, tag="scps")
                            for e in range(2):
                                psl = slice(64 * e, 64 * (e + 1))
                                nc.tensor.matmul(
                                    ps[:, e * 512 + c0:(e + 1) * 512],
                                    kT_sb[psl, qoff + jt * 128: qoff + (jt + 1) * 128],
                                    qT_sb[psl, qoff + qoffc + c0: qoff + qoffc + 512],
                                    start=True, stop=True)
                            if prev is not None:
                                pjt, pp = prev
                                for e in range(2):
                                    h = 2 * hp + e
                                    nc.tensor.matmul(
                                        pav[e][0:65, :],
                                        v_sb[:, pjt * 260 + h * 65: pjt * 260 + (h + 1) * 65],
                                        pp[:, e * 512:(e + 1) * 512],
                                        start=(pjt == 0), stop=False)
                            if jt == 2 and pending:
                                pending.pop(0)()
                                pending.pop(0)()
                            p = probsp.tile([128, 1024], F16, tag="p")
                            p3 = p[:].rearrange("p (b c) -> p b c", b=2)
                            ps3 = ps[:].rearrange("p (b c) -> p b c", b=2)
                            if c0 > 0:
                                nc.vector.memset(p3[:, :, 0:c0], 0.0)
                            nc.scalar.activation(
                                p3[:, :, c0:512], ps3[:, :, c0:512],
                                mybir.ActivationFunctionType.Exp,
                                scale=SCALE)
                            if r >= 0:
                                nc.vector.tensor_mul(
                                    p3[:, :, c0:c0 + 128],
                                    p3[:, :, c0:c0 + 128],
                                    mask3[:, :, 0:128])
                            prev = (jt, p)
                        pjt, pp = prev
                        for e in range(2):
                            h = 2 * hp + e
                            nc.tensor.matmul(
                                pav[e][0:65, :],
                                v_sb[:, pjt * 260 + h * 65: pjt * 260 + (h + 1) * 65],
                                pp[:, e * 512:(e + 1) * 512],
                                start=(pjt == 0), stop=True)
                        # denominators: free the PSUM accumulators fast,
                        # then normalize via broadcast off the PE path
                        for e in range(2):
                            outu = outup.tile([128, 512], F16, name="outu",
                                              tag=f"outu{e}")
                            nc.vector.tensor_copy(outu[0:65, :], pav[e][0:65, :])
                            rr = rrp.tile([1, 512], F16, name="rr", tag=f"rr{e}")
                            with nc.allow_low_precision(
                                    reason="f16 softmax denom; tol is 2e-2"):
                                nc.vector.reciprocal(rr[0:1, :], outu[64:65, :])
                            db = binvp.tile([64, 512], F16, name="db",
                                            tag=f"db{e}")
                            nc.gpsimd.partition_broadcast(db[0:64, :], rr[0:1, :])
                            nc.vector.tensor_mul(
                                outT_sb[64 * e:64 * (e + 1),
                                        qoff + qoffc: qoff + qoffc + 512],
                                outu[0:64, :], db[0:64, :])
                        if pending:
                            pending.pop(0)()
                            pending.pop(0)()
                    assert not pending
                    pending = make_units(ic)
                # tail: the last chunk's output projection
                for fn in pending:
                    fn()

    nc.compile()
    return nc


def _rope_tables():
    inv_freq = 1.0 / (ROPE_BASE ** (np.arange(0, HD, 2, dtype=np.float64) / HD))
    t = np.arange(S, dtype=np.float64)
    freqs = np.outer(t, inv_freq)                      # [S, hd/2]
    emb = np.concatenate([freqs, freqs], axis=-1)      # [S, hd]
    cosT = np.cos(emb).T.astype(np.float32)            # [hd, S]
    sinT = np.sin(emb).T.astype(np.float32)
    cos2 = np.vstack([cosT, cosT])                     # [128, S]
    sin2 = np.vstack([sinT, sinT])
    return np.ascontiguousarray(cos2), np.ascontiguousarray(sin2)


def _rot_matrix():
    r = np.zeros((HD, HD), dtype=np.float32)
    half = HD // 2
    for d in range(half):
        r[d, d + half] = -1.0       # rot(q)[0:32] = -q[32:64]
        r[d + half, d] = 1.0        # rot(q)[32:64] = q[0:32]
    r2 = np.zeros((128, 128), dtype=np.float32)
    r2[0:HD, 0:HD] = r
    r2[HD:128, HD:128] = r
    return np.ascontiguousarray(r2.T)


def _mask_tile():
    # [128, 256]: the same lower-triangle-of-the-diagonal-128-block twice
    # (so a [128, 2, 128] view multiplies both heads of a pair at once)
    jl = np.arange(128)[:, None]
    il = np.arange(128)[None, :]
    tri = (jl <= il).astype(np.float32)
    return np.ascontiguousarray(np.concatenate([tri, tri], axis=1))


def _tile_rows(a):
    """[K*128, C] -> [128, K*C] with row r of tile k at partition r%...:
    a[k*128 + p, :] lands at [p, k*C : (k+1)*C]."""
    kk = a.shape[0] // 128
    return np.ascontiguousarray(
        a.reshape(kk, 128, a.shape[1]).transpose(1, 0, 2).reshape(128, -1))


_prog_cache = {}

# test harness hooks: set TRACE=True before calling kernel() to capture an
# NTFF profile; the BassKernelResults lands in LAST_RESULTS.
TRACE = False
LAST_RESULTS = None


def _f16(a):
    return np.ascontiguousarray(a.astype(np.float16))


def kernel(x, w_qkv, w_out, mask):
    x = np.asarray(x, dtype=np.float32)
    w_qkv = np.asarray(w_qkv, dtype=np.float32)
    w_out = np.asarray(w_out, dtype=np.float32)

    if "nc" not in _prog_cache:
        _prog_cache["nc"] = _build_program()
    nc = _prog_cache["nc"]

    cos2, sin2 = _rope_tables()
    rmatT = _rot_matrix()
    mask2 = _mask_tile()

    in_maps = []
    for c in range(N_CORES):
        b = c // 4
        g = c % 4
        cw = HEADS_PER_CORE * HD   # 256
        wq = w_qkv[:, g * cw:(g + 1) * cw]
        wk = w_qkv[:, D + g * cw: D + (g + 1) * cw]
        wv = w_qkv[:, 2 * D + g * cw: 2 * D + (g + 1) * cw]
        w_c = np.concatenate([wq, wk, wv], axis=1)
        wo_c = w_out[g * cw:(g + 1) * cw, :]
        xT_c = x[b].T
        in_maps.append({
            "xT": _f16(_tile_rows(xT_c)), "w": _f16(_tile_rows(w_c)),
            "wo": _f16(_tile_rows(wo_c)),
            "cosT": _f16(cos2), "sinT": _f16(sin2),
            "rmatT": _f16(rmatT), "mask2": _f16(mask2),
        })

    res = run_bass_kernel_spmd(nc, in_maps, list(range(N_CORES)),
                               trace=TRACE)
    global LAST_RESULTS
    LAST_RESULTS = res
    y = np.zeros((B, S, D), dtype=np.float32)
    for c in range(N_CORES):
        y[c // 4] += res.results[c]["y"].astype(np.float32)
    return y


# revision 14
# speedup vs baseline: 1.2143x; 1.2143x over previous
"""Causal self-attention (B=2, S=2048, D=1024, H=16, hd=64) on 8 TRN2 NeuronCores.

Sharding: batch x head-group. Core c handles batch c//4 and heads
4*(c%4) .. 4*(c%4)+3. Each core computes its 4 heads' attention plus the
partial output projection; the host sums the 4 partial projections per batch.

v2 (vs the 239us baseline):
  - inputs host-pretiled to [128, K*cols] so each tensor loads with one
    contiguous-per-partition DMA; DMAs spread over 4 engine queues so the
    ~1us SWDGE descriptor-gen per dma_start parallelizes (compute starts
    ~3us instead of ~28us).
  - gpsimd ISA library preloaded with a dummy partition_broadcast at t=0
    (the lazy lib load cost ~7us on the first chunk's denominator chain).
  - scores / exp / mask exploit causality inside the diagonal 512-chunk:
    cols < 128*r of a diagonal key-tile are skipped (matmul + exp trimmed,
    probs zero-memset), the 0/1 mask multiply shrinks to the [128,128]
    triangle. Exp for the head pair is one [128, 2, cols] instruction.
  - denominator chain per (hp,e): copy PSUM->SBUF f16 (frees the PSUM
    accumulator ~0.6us after the last AV), reciprocal of the sum row,
    gpsimd partition_broadcast, one f16 multiply. avps needs only 2 banks.
  - output projection of chunk i is emitted inside chunk i+1's score loop
    (PE filler while Act runs exp), chunks processed in order 0,3,2,1 so
    the serial tail is the smallest chunk; y stored f16, one DMA per chunk.
"""

import sys

try:
    import concourse.bass  # noqa: F401
except ImportError:
    sys.path.insert(0, "/opt/trn_rl_repo")

import numpy as np
import concourse.bacc as bacc
import concourse.mybir as mybir
from concourse.tile import TileContext
from concourse.bass_utils import run_bass_kernel_spmd

F32 = mybir.dt.float32
F16 = mybir.dt.float16

B, S, D = 2, 2048, 1024
H, HD = 16, 64
HEADS_PER_CORE = 4
N_CORES = 8
ROPE_BASE = 10000.0
SCALE = HD ** -0.5

KT = D // 128          # 8  contraction tiles for the QKV projection
ST = S // 128          # 16 sequence tiles of 128
NC_CH = S // 512       # 4  sequence chunks of 512
WF = 3 * HEADS_PER_CORE * HD   # 768 projection features per core
VOFF = 2 * HEADS_PER_CORE * HD # 512 column offset of the v block in w

CHUNK_ORDER = [0, 3, 2, 1]


def _build_program():
    nc = bacc.Bacc("TRN2", target_bir_lowering=False, debug=False,
                   num_devices=N_CORES)

    xT = nc.dram_tensor("xT", [128, KT * S], F16, kind="ExternalInput")
    w = nc.dram_tensor("w", [128, KT * WF], F16, kind="ExternalInput")
    wo = nc.dram_tensor("wo", [128, 2 * D], F16, kind="ExternalInput")
    cosT = nc.dram_tensor("cosT", [128, S], F16, kind="ExternalInput")
    sinT = nc.dram_tensor("sinT", [128, S], F16, kind="ExternalInput")
    rmatT = nc.dram_tensor("rmatT", [128, 128], F16, kind="ExternalInput")
    mask2 = nc.dram_tensor("mask2", [128, 256], F16, kind="ExternalInput")
    y = nc.dram_tensor("y", [S, D], F16, kind="ExternalOutput")

    with TileContext(nc) as tc:
        with (
            tc.tile_pool(name="const", bufs=1) as constp,
            tc.tile_pool(name="acts", bufs=1) as actsp,
        ):
            w_sb = constp.tile([128, KT * WF], F16)
            wo_sb = constp.tile([128, 2 * D], F16)
            cos_sb = constp.tile([128, S], F16)
            sin_sb = constp.tile([128, S], F16)
            rmat_sb = constp.tile([128, 128], F16)
            mask_sb = constp.tile([128, 256], F16)
            warm_sb = constp.tile([128, 8], F16)

            # gpsimd ISA library preload: a dummy broadcast at t=0 so the
            # ~7us lazy lib load overlaps the input DMAs.
            nc.vector.memset(warm_sb[0:1, :], 1.0)
            nc.gpsimd.partition_broadcast(warm_sb[64:128, :], warm_sb[0:1, :])

            # input DMAs: only SP/Act (HWDGE) and gpsimd (SWDGE) can issue.
            # x tiles on sync, w tiles + small constants on scalar, bulky
            # late-needed constants on gpsimd (queued behind the lib load).

            # activations produced by the QKV phase, consumed by attention
            qT_sb = actsp.tile([128, 2 * S], F16)   # head pairs 0|1
            kT_sb = actsp.tile([128, 2 * S], F16)
            v_sb = actsp.tile([128, ST * 260], F16) # 16 seq tiles x 4x65
            # per-chunk normalized attention output [d(2 heads), hp*512+q].
            # One tile per chunk so the deferred output projection of chunk
            # i never picks up a (coarse-tracked) dependency on chunk i+1's
            # writes.
            outTc = [actsp.tile([128, 1024], F16, name=f"outT{_c}")
                     for _c in range(NC_CH)]

            # ones columns of the v blocks (col 64 of each 65-block)
            ones_cols = v_sb[:, 0:ST * 260].rearrange(
                "p (b c) -> p b c", c=65)[:, :, 64:65]
            nc.vector.memset(ones_cols, 1.0)

            # ---------------- QKV projection + RoPE ----------------
            with (
                tc.tile_pool(name="xt", bufs=1) as xtp,
                tc.tile_pool(name="qkps", bufs=4, space="PSUM") as qkps,
                tc.tile_pool(name="rotps", bufs=2, space="PSUM") as rotps,
                tc.tile_pool(name="vps", bufs=2, space="PSUM") as vps,
                tc.tile_pool(name="qpre", bufs=2) as qprep,
                tc.tile_pool(name="ropet", bufs=2) as ropetp,
            ):
                xT_sb = xtp.tile([128, KT * S], F16)
                # k ascending so the mt=0 accumulation paces with arrivals
                for k in range(KT):
                    nc.sync.dma_start(
                        xT_sb[:, k * S:(k + 1) * S], xT[:, k * S:(k + 1) * S])
                    nc.scalar.dma_start(
                        w_sb[:, k * WF:(k + 1) * WF], w[:, k * WF:(k + 1) * WF])
                nc.scalar.dma_start(rmat_sb[:], rmatT[:])
                nc.scalar.dma_start(mask_sb[:], mask2[:])
                nc.gpsimd.dma_start(cos_sb[:], cosT[:])
                nc.gpsimd.dma_start(sin_sb[:], sinT[:])
                nc.gpsimd.dma_start(wo_sb[:], wo[:])

                # q/k head-pair tiles: mt 0,1 -> q pairs; 2,3 -> k pairs
                for mt in range(4):
                    dest = qT_sb if mt < 2 else kT_sb
                    doff = (mt % 2) * S
                    pts = [qkps.tile([128, 512], F32, name=f"qkpsum{_n}",
                                     tag="qkpsum") for _n in range(NC_CH)]
                    for k in range(KT):
                        lhsT = w_sb[:, k * WF + mt * 128: k * WF + (mt + 1) * 128]
                        for n in range(NC_CH):
                            nc.tensor.matmul(
                                pts[n][:],
                                lhsT,
                                xT_sb[:, k * S + n * 512: k * S + (n + 1) * 512],
                                start=(k == 0), stop=(k == KT - 1))
                    for n in range(NC_CH):
                        qpre = qprep.tile([128, 512], F16)
                        nc.scalar.copy(qpre[:], pts[n][:])
                        rot = rotps.tile([128, 512], F32)
                        nc.tensor.matmul(rot[:], rmat_sb[:], qpre[:],
                                         start=True, stop=True)
                        t1 = ropetp.tile([128, 512], F16, tag="t1")
                        t2 = ropetp.tile([128, 512], F16, tag="t2")
                        nc.vector.tensor_mul(
                            t1[:], qpre[:], cos_sb[:, n * 512:(n + 1) * 512])
                        nc.vector.tensor_mul(
                            t2[:], rot[:], sin_sb[:, n * 512:(n + 1) * 512])
                        nc.vector.tensor_add(
                            dest[:, doff + n * 512: doff + (n + 1) * 512],
                            t1[:], t2[:])

                # v in [seq, head-block] layout
                for st in range(ST):
                    pv = vps.tile([128, 256], F32)
                    for k in range(KT):
                        nc.tensor.matmul(
                            pv[:],
                            xT_sb[:, k * S + st * 128: k * S + (st + 1) * 128],
                            w_sb[:, k * WF + VOFF: k * WF + WF],
                            start=(k == 0), stop=(k == KT - 1))
                    vdst = v_sb[:, st * 260:(st + 1) * 260].rearrange(
                        "p (h c) -> p h c", c=65)[:, :, 0:64]
                    nc.vector.tensor_copy(
                        vdst, pv[:].rearrange("p (h c) -> p h c", c=64))

            # ---------------- attention + output projection ----------------
            with (
                tc.tile_pool(name="scps", bufs=2, space="PSUM") as scps,
                tc.tile_pool(name="avps", bufs=1, space="PSUM") as avps,
                tc.tile_pool(name="yps", bufs=2, space="PSUM") as yps,
                tc.tile_pool(name="probs", bufs=4) as probsp,
                tc.tile_pool(name="outu", bufs=2) as outup,
                tc.tile_pool(name="rrp", bufs=2) as rrp,
                tc.tile_pool(name="binv", bufs=2) as binvp,
                tc.tile_pool(name="ysb", bufs=2) as ysbp,
            ):
                mask3 = mask_sb[:, 0:256].rearrange("p (b c) -> p b c", b=2)

                # deferred output-projection units; each unit is one
                # (st, nn) pair: 2 accumulating matmuls + a PSUM->SBUF f16
                # copy into the chunk's staging tile, one DMA per chunk.
                pending = []   # list of closures for the previous chunk

                def make_units(pc):
                    ycb = {}

                    def unit(u, pc=pc, ycb=ycb):
                        if u == 0:
                            ycb["t"] = ysbp.tile([128, 4096], F16, name="ycb",
                                                 tag="ycb")
                        sti, nn = u // 2, u % 2
                        py = yps.tile([128, 512], F32, name="py", tag="py")
                        for hp2 in range(2):
                            nc.tensor.matmul(
                                py[:],
                                outTc[pc][:, hp2 * 512 + sti * 128: hp2 * 512 + (sti + 1) * 128],
                                wo_sb[:, hp2 * D + nn * 512: hp2 * D + (nn + 1) * 512],
                                start=(hp2 == 0), stop=(hp2 == 1))
                        nc.vector.tensor_copy(
                            ycb["t"][:, sti * 1024 + nn * 512: sti * 1024 + (nn + 1) * 512],
                            py[:])
                        if u == 7:
                            dst = y[pc * 512:(pc + 1) * 512, :].rearrange(
                                "(s p) d -> p s d", p=128)
                            nc.sync.dma_start(
                                dst, ycb["t"][:].rearrange("p (s d) -> p s d", s=4))
                    return [lambda u=u: unit(u) for u in range(8)]

                for ic in CHUNK_ORDER:
                    jmax = 4 * ic + 4
                    qoffc = ic * 512
                    for hp in range(2):
                        qoff = hp * S
                        pav = [avps.tile([128, 512], F32, name=f"av{e}",
                                         tag=f"av{e}") for e in range(2)]
                        prev = None   # (jt, probs tile)
                        for jt in range(jmax):
                            r = jt - 4 * ic
                            c0 = 128 * r if r > 0 else 0
                            ps = scps.tile([128, 1024], F32, tag="scps")
                            for e in range(2):
                                psl = slice(64 * e, 64 * (e + 1))
                                nc.tensor.matmul(
                                    ps[:, e * 512 + c0:(e + 1) * 512],
                                    kT_sb[psl, qoff + jt * 128: qoff + (jt + 1) * 128],
                                    qT_sb[psl, qoff + qoffc + c0: qoff + qoffc + 512],
                                    start=True, stop=True)
                            if prev is not None:
                                pjt, pp = prev
                                for e in range(2):
                                    h = 2 * hp + e
                                    nc.tensor.matmul(
                                        pav[e][0:65, :],
                                        v_sb[:, pjt * 260 + h * 65: pjt * 260 + (h + 1) * 65],
                                        pp[:, e * 512:(e + 1) * 512],
                                        start=(pjt == 0), stop=False)
                            if jt == 2 and pending:
                                pending.pop(0)()
                                pending.pop(0)()
                            p = probsp.tile([128, 1024], F16, tag="p")
                            p3 = p[:].rearrange("p (b c) -> p b c", b=2)
                            ps3 = ps[:].rearrange("p (b c) -> p b c", b=2)
                            if c0 > 0:
                                nc.vector.memset(p3[:, :, 0:c0], 0.0)
                            nc.scalar.activation(
                                p3[:, :, c0:512], ps3[:, :, c0:512],
                                mybir.ActivationFunctionType.Exp,
                                scale=SCALE)
                            if r >= 0:
                                nc.vector.tensor_mul(
                                    p3[:, :, c0:c0 + 128],
                                    p3[:, :, c0:c0 + 128],
                                    mask3[:, :, 0:128])
                            prev = (jt, p)
                        pjt, pp = prev
                        for e in range(2):
                            h = 2 * hp + e
                            nc.tensor.matmul(
                                pav[e][0:65, :],
                                v_sb[:, pjt * 260 + h * 65: pjt * 260 + (h + 1) * 65],
                                pp[:, e * 512:(e + 1) * 512],
                                start=(pjt == 0), stop=True)
                        # denominators: free the PSUM accumulators fast,
                        # then normalize via broadcast off the PE path
                        for e in range(2):
                            outu = outup.tile([128, 512], F16, name="outu",
                                              tag=f"outu{e}")
                            nc.vector.tensor_copy(outu[0:64, :], pav[e][0:64, :])
                            dr = rrp.tile([1, 512], F32, name="dr", tag=f"dr{e}")
                            nc.vector.tensor_copy(dr[0:1, :], pav[e][64:65, :])
                            rr = rrp.tile([1, 512], F32, name="rr", tag=f"rr{e}")
                            nc.vector.reciprocal_approx_fast(
                                rr[0:1, :], dr[0:1, :])
                            db = binvp.tile([64, 512], F32, name="db",
                                            tag=f"db{e}")
                            nc.gpsimd.partition_broadcast(db[0:64, :], rr[0:1, :])
                            nc.vector.tensor_mul(
                                outTc[ic][64 * e:64 * (e + 1),
                                          hp * 512: hp * 512 + 512],
                                outu[0:64, :], db[0:64, :])
                        if pending:
                            pending.pop(0)()
                            pending.pop(0)()
                    assert not pending
                    pending = make_units(ic)
                # tail: the last chunk's output projection
                for fn in pending:
                    fn()

    nc.compile()
    return nc


def _rope_tables():
    inv_freq = 1.0 / (ROPE_BASE ** (np.arange(0, HD, 2, dtype=np.float64) / HD))
    t = np.arange(S, dtype=np.float64)
    freqs = np.outer(t, inv_freq)                      # [S, hd/2]
    emb = np.concatenate([freqs, freqs], axis=-1)      # [S, hd]
    cosT = np.cos(emb).T.astype(np.float32)            # [hd, S]
    sinT = np.sin(emb).T.astype(np.float32)
    cos2 = np.vstack([cosT, cosT])                     # [128, S]
    sin2 = np.vstack([sinT, sinT])
    return np.ascontiguousarray(cos2), np.ascontiguousarray(sin2)


def _rot_matrix():
    r = np.zeros((HD, HD), dtype=np.float32)
    half = HD // 2
    for d in range(half):
        r[d, d + half] = -1.0       # rot(q)[0:32] = -q[32:64]
        r[d + half, d] = 1.0        # rot(q)[32:64] = q[0:32]
    r2 = np.zeros((128, 128), dtype=np.float32)
    r2[0:HD, 0:HD] = r
    r2[HD:128, HD:128] = r
    return np.ascontiguousarray(r2.T)


def _mask_tile():
    # [128, 256]: the same lower-triangle-of-the-diagonal-128-block twice
    # (so a [128, 2, 128] view multiplies both heads of a pair at once)
    jl = np.arange(128)[:, None]
    il = np.arange(128)[None, :]
    tri = (jl <= il).astype(np.float32)
    return np.ascontiguousarray(np.concatenate([tri, tri], axis=1))


def _tile_rows(a):
    """[K*128, C] -> [128, K*C] with row r of tile k at partition r%...:
    a[k*128 + p, :] lands at [p, k*C : (k+1)*C]."""
    kk = a.shape[0] // 128
    return np.ascontiguousarray(
        a.reshape(kk, 128, a.shape[1]).transpose(1, 0, 2).reshape(128, -1))


_prog_cache = {}

# test harness hooks: set TRACE=True before calling kernel() to capture an
# NTFF profile; the BassKernelResults lands in LAST_RESULTS.
TRACE = False
LAST_RESULTS = None


def _f16(a):
    return np.ascontiguousarray(a.astype(np.float16))


def kernel(x, w_qkv, w_out, mask):
    x = np.asarray(x, dtype=np.float32)
    w_qkv = np.asarray(w_qkv, dtype=np.float32)
    w_out = np.asarray(w_out, dtype=np.float32)

    if "nc" not in _prog_cache:
        _prog_cache["nc"] = _build_program()
    nc = _prog_cache["nc"]

    cos2, sin2 = _rope_tables()
    rmatT = _rot_matrix()
    mask2 = _mask_tile()

    in_maps = []
    for c in range(N_CORES):
        b = c // 4
        g = c % 4
        cw = HEADS_PER_CORE * HD   # 256
        wq = w_qkv[:, g * cw:(g + 1) * cw]
        wk = w_qkv[:, D + g * cw: D + (g + 1) * cw]
        wv = w_qkv[:, 2 * D + g * cw: 2 * D + (g + 1) * cw]
        w_c = np.concatenate([wq, wk, wv], axis=1)
        wo_c = w_out[g * cw:(g + 1) * cw, :]
        xT_c = x[b].T
        in_maps.append({
            "xT": _f16(_tile_rows(xT_c)), "w": _f16(_tile_rows(w_c)),
            "wo": _f16(_tile_rows(wo_c)),
            "cosT": _f16(cos2), "sinT": _f16(sin2),
            "rmatT": _f16(rmatT), "mask2": _f16(mask2),
        })

    res = run_bass_kernel_spmd(nc, in_maps, list(range(N_CORES)),
                               trace=TRACE)
    global LAST_RESULTS
    LAST_RESULTS = res
    y = np.zeros((B, S, D), dtype=np.float32)
    for c in range(N_CORES):
        y[c // 4] += res.results[c]["y"].astype(np.float32)
    return y


# revision 18
# speedup vs baseline: 1.2285x; 1.0117x over previous
"""Causal self-attention (B=2, S=2048, D=1024, H=16, hd=64) on 8 TRN2 NeuronCores.

Sharding: batch x head-group. Core c handles batch c//4 and heads
4*(c%4) .. 4*(c%4)+3. Each core computes its 4 heads' attention plus the
partial output projection; the host sums the 4 partial projections per batch.

v2 (vs the 239us baseline):
  - inputs host-pretiled to [128, K*cols] so each tensor loads with one
    contiguous-per-partition DMA; DMAs spread over 4 engine queues so the
    ~1us SWDGE descriptor-gen per dma_start parallelizes (compute starts
    ~3us instead of ~28us).
  - gpsimd ISA library preloaded with a dummy partition_broadcast at t=0
    (the lazy lib load cost ~7us on the first chunk's denominator chain).
  - scores / exp / mask exploit causality inside the diagonal 512-chunk:
    cols < 128*r of a diagonal key-tile are skipped (matmul + exp trimmed,
    probs zero-memset), the 0/1 mask multiply shrinks to the [128,128]
    triangle. Exp for the head pair is one [128, 2, cols] instruction.
  - denominator chain per (hp,e): copy PSUM->SBUF f16 (frees the PSUM
    accumulator ~0.6us after the last AV), reciprocal of the sum row,
    gpsimd partition_broadcast, one f16 multiply. avps needs only 2 banks.
  - output projection of chunk i is emitted inside chunk i+1's score loop
    (PE filler while Act runs exp), chunks processed in order 0,3,2,1 so
    the serial tail is the smallest chunk; y stored f16, one DMA per chunk.
"""

import sys

try:
    import concourse.bass  # noqa: F401
except ImportError:
    sys.path.insert(0, "/opt/trn_rl_repo")

import numpy as np
import concourse.bacc as bacc
import concourse.mybir as mybir
from concourse.tile import TileContext
from concourse.bass_utils import run_bass_kernel_spmd

F32 = mybir.dt.float32
F16 = mybir.dt.float16

B, S, D = 2, 2048, 1024
H, HD = 16, 64
HEADS_PER_CORE = 4
N_CORES = 8
ROPE_BASE = 10000.0
SCALE = HD ** -0.5

KT = D // 128          # 8  contraction tiles for the QKV projection
ST = S // 128          # 16 sequence tiles of 128
NC_CH = S // 512       # 4  sequence chunks of 512
WF = 3 * HEADS_PER_CORE * HD   # 768 projection features per core
VOFF = 2 * HEADS_PER_CORE * HD # 512 column offset of the v block in w

CHUNK_ORDER = [1, 3, 2, 0]


def _build_program():
    nc = bacc.Bacc("TRN2", target_bir_lowering=False, debug=False,
                   num_devices=N_CORES)

    xT = nc.dram_tensor("xT", [128, KT * S], F16, kind="ExternalInput")
    w = nc.dram_tensor("w", [128, KT * WF], F16, kind="ExternalInput")
    wo = nc.dram_tensor("wo", [128, 2 * D], F16, kind="ExternalInput")
    cosT = nc.dram_tensor("cosT", [128, S], F16, kind="ExternalInput")
    sinT = nc.dram_tensor("sinT", [128, S], F16, kind="ExternalInput")
    rmatT = nc.dram_tensor("rmatT", [128, 128], F16, kind="ExternalInput")
    mask2 = nc.dram_tensor("mask2", [128, 256], F16, kind="ExternalInput")
    y = nc.dram_tensor("y", [S, D], F16, kind="ExternalOutput")

    with TileContext(nc) as tc:
        with (
            tc.tile_pool(name="const", bufs=1) as constp,
            tc.tile_pool(name="acts", bufs=1) as actsp,
        ):
            w_sb = constp.tile([128, KT * WF], F16)
            wo_sb = constp.tile([128, 2 * D], F16)
            cos_sb = constp.tile([128, S], F16)
            sin_sb = constp.tile([128, S], F16)
            rmat_sb = constp.tile([128, 128], F16)
            mask_sb = constp.tile([128, 256], F16)
            warm_sb = constp.tile([128, 8], F16)

            # gpsimd ISA library preload: a dummy broadcast at t=0 so the
            # ~7us lazy lib load overlaps the input DMAs.
            nc.vector.memset(warm_sb[0:1, :], 1.0)
            nc.gpsimd.partition_broadcast(warm_sb[64:128, :], warm_sb[0:1, :])

            # input DMAs: only SP/Act (HWDGE) and gpsimd (SWDGE) can issue.
            # x tiles on sync, w tiles + small constants on scalar, bulky
            # late-needed constants on gpsimd (queued behind the lib load).

            # activations produced by the QKV phase, consumed by attention
            qT_sb = actsp.tile([128, 2 * S], F16)   # head pairs 0|1
            kT_sb = actsp.tile([128, 2 * S], F16)
            v_sb = actsp.tile([128, ST * 260], F16) # 16 seq tiles x 4x65
            # per-chunk normalized attention output [d(2 heads), hp*512+q].
            # One tile per chunk so the deferred output projection of chunk
            # i never picks up a (coarse-tracked) dependency on chunk i+1's
            # writes.
            outTc = [actsp.tile([128, 1024], F16, name=f"outT{_c}")
                     for _c in range(NC_CH)]

            # ones columns of the v blocks (col 64 of each 65-block)
            ones_cols = v_sb[:, 0:ST * 260].rearrange(
                "p (b c) -> p b c", c=65)[:, :, 64:65]
            nc.vector.memset(ones_cols, 1.0)

            # ---------------- QKV projection + RoPE ----------------
            with (
                tc.tile_pool(name="xt", bufs=1) as xtp,
                tc.tile_pool(name="qkps", bufs=4, space="PSUM") as qkps,
                tc.tile_pool(name="rotps", bufs=2, space="PSUM") as rotps,
                tc.tile_pool(name="vps", bufs=2, space="PSUM") as vps,
                tc.tile_pool(name="qpre", bufs=2) as qprep,
                tc.tile_pool(name="ropet", bufs=2) as ropetp,
            ):
                xT_sb = xtp.tile([128, KT * S], F16)
                # k ascending so the mt=0 accumulation paces with arrivals;
                # x/w alternate between the two HWDGE queues so tile k needs
                # only ~k transfers on each queue before it lands
                for k in range(KT):
                    qa, qb = (nc.sync, nc.scalar) if k % 2 == 0 else (nc.scalar, nc.sync)
                    qa.dma_start(
                        xT_sb[:, k * S:(k + 1) * S], xT[:, k * S:(k + 1) * S])
                    qb.dma_start(
                        w_sb[:, k * WF:(k + 1) * WF], w[:, k * WF:(k + 1) * WF])
                nc.gpsimd.dma_start(rmat_sb[:], rmatT[:])
                nc.gpsimd.dma_start(cos_sb[:], cosT[:])
                nc.gpsimd.dma_start(sin_sb[:], sinT[:])
                nc.gpsimd.dma_start(wo_sb[:], wo[:])
                nc.gpsimd.dma_start(mask_sb[:], mask2[:])

                # q/k head-pair tiles: mt 0,1 -> q pairs; 2,3 -> k pairs
                for mt in range(4):
                    dest = qT_sb if mt < 2 else kT_sb
                    doff = (mt % 2) * S
                    pts = [qkps.tile([128, 512], F32, name=f"qkpsum{_n}",
                                     tag="qkpsum") for _n in range(NC_CH)]
                    for k in range(KT):
                        lhsT = w_sb[:, k * WF + mt * 128: k * WF + (mt + 1) * 128]
                        for n in range(NC_CH):
                            nc.tensor.matmul(
                                pts[n][:],
                                lhsT,
                                xT_sb[:, k * S + n * 512: k * S + (n + 1) * 512],
                                start=(k == 0), stop=(k == KT - 1))
                    for n in range(NC_CH):
                        qpre = qprep.tile([128, 512], F16)
                        nc.scalar.copy(qpre[:], pts[n][:])
                        rot = rotps.tile([128, 512], F32)
                        nc.tensor.matmul(rot[:], rmat_sb[:], qpre[:],
                                         start=True, stop=True)
                        t1 = ropetp.tile([128, 512], F16, tag="t1")
                        t2 = ropetp.tile([128, 512], F16, tag="t2")
                        nc.vector.tensor_mul(
                            t1[:], qpre[:], cos_sb[:, n * 512:(n + 1) * 512])
                        nc.vector.tensor_mul(
                            t2[:], rot[:], sin_sb[:, n * 512:(n + 1) * 512])
                        nc.vector.tensor_add(
                            dest[:, doff + n * 512: doff + (n + 1) * 512],
                            t1[:], t2[:])

                # v in [seq, head-block] layout
                for st in range(ST):
                    pv = vps.tile([128, 256], F32)
                    for k in range(KT):
                        nc.tensor.matmul(
                            pv[:],
                            xT_sb[:, k * S + st * 128: k * S + (st + 1) * 128],
                            w_sb[:, k * WF + VOFF: k * WF + WF],
                            start=(k == 0), stop=(k == KT - 1))
                    vdst = v_sb[:, st * 260:(st + 1) * 260].rearrange(
                        "p (h c) -> p h c", c=65)[:, :, 0:64]
                    nc.vector.tensor_copy(
                        vdst, pv[:].rearrange("p (h c) -> p h c", c=64))

            # ---------------- attention + output projection ----------------
            with (
                tc.tile_pool(name="scps", bufs=2, space="PSUM") as scps,
                tc.tile_pool(name="avps", bufs=1, space="PSUM") as avps,
                tc.tile_pool(name="yps", bufs=2, space="PSUM") as yps,
                tc.tile_pool(name="probs", bufs=5) as probsp,
                tc.tile_pool(name="outu", bufs=2) as outup,
                tc.tile_pool(name="rrp", bufs=2) as rrp,
                tc.tile_pool(name="binv", bufs=2) as binvp,
                tc.tile_pool(name="ysb", bufs=2) as ysbp,
            ):
                mask3 = mask_sb[:, 0:256].rearrange("p (b c) -> p b c", b=2)

                # deferred output-projection units; each unit is one
                # (st, nn) pair: 2 accumulating matmuls + a PSUM->SBUF f16
                # copy into the staging tile; one DMA per seq tile (so the
                # final DMA of the kernel is only 256KB). The last chunk's
                # staging copies go on the Act engine (idle once exps end).
                pending = []   # list of closures for the previous chunk

                def make_units(pc, last=False):
                    ycb = {}

                    def unit(u, pc=pc, ycb=ycb):
                        if u == 0:
                            ycb["t"] = ysbp.tile([128, 4096], F16, name="ycb",
                                                 tag="ycb")
                        sti, nn = u // 2, u % 2
                        py = yps.tile([128, 512], F32, name="py", tag="py")
                        for hp2 in range(2):
                            nc.tensor.matmul(
                                py[:],
                                outTc[pc][:, hp2 * 512 + sti * 128: hp2 * 512 + (sti + 1) * 128],
                                wo_sb[:, hp2 * D + nn * 512: hp2 * D + (nn + 1) * 512],
                                start=(hp2 == 0), stop=(hp2 == 1))
                        ycs = ycb["t"][:, sti * 1024 + nn * 512:
                                       sti * 1024 + (nn + 1) * 512]
                        if last:
                            nc.scalar.copy(ycs, py[:])
                        else:
                            nc.vector.tensor_copy(ycs, py[:])
                        if nn == 1:
                            st = pc * 4 + sti
                            nc.sync.dma_start(
                                y[st * 128:(st + 1) * 128, :],
                                ycb["t"][:, sti * 1024:(sti + 1) * 1024])
                    return [lambda u=u: unit(u) for u in range(8)]

                def emit_av(hp, pav, jt, pp, stop):
                    for e in range(2):
                        h = 2 * hp + e
                        nc.tensor.matmul(
                            pav[e][0:65, :],
                            v_sb[:, jt * 260 + h * 65: jt * 260 + (h + 1) * 65],
                            pp[:, e * 512:(e + 1) * 512],
                            start=(jt == 0), stop=stop)

                for ci, ic in enumerate(CHUNK_ORDER):
                    jmax = 4 * ic + 4
                    qoffc = ic * 512
                    for hp in range(2):
                        qoff = hp * S
                        pav = [avps.tile([128, 512], F32, name=f"av{e}",
                                         tag=f"av{e}") for e in range(2)]
                        pipe = []   # (jt, probs tile), AV runs 2 jts behind
                        for jt in range(jmax):
                            r = jt - 4 * ic
                            c0 = 128 * r if r > 0 else 0
                            ps = scps.tile([128, 1024], F32, tag="scps")
                            for e in range(2):
                                psl = slice(64 * e, 64 * (e + 1))
                                nc.tensor.matmul(
                                    ps[:, e * 512 + c0:(e + 1) * 512],
                                    kT_sb[psl, qoff + jt * 128: qoff + (jt + 1) * 128],
                                    qT_sb[psl, qoff + qoffc + c0: qoff + qoffc + 512],
                                    start=True, stop=True)
                            if len(pipe) >= 2:
                                emit_av(hp, pav, *pipe.pop(0), stop=False)
                            # output-projection filler keeps the PE ramped
                            # while Act paces the exp pipeline
                            if jt % 3 == 2 and pending:
                                pending.pop(0)()
                            p = probsp.tile([128, 1024], F16, tag="p")
                            p3 = p[:].rearrange("p (b c) -> p b c", b=2)
                            ps3 = ps[:].rearrange("p (b c) -> p b c", b=2)
                            if c0 > 0:
                                nc.vector.memset(p3[:, :, 0:c0], 0.0)
                            nc.scalar.activation(
                                p3[:, :, c0:512], ps3[:, :, c0:512],
                                mybir.ActivationFunctionType.Exp,
                                scale=SCALE)
                            if r >= 0:
                                nc.vector.tensor_mul(
                                    p3[:, :, c0:c0 + 128],
                                    p3[:, :, c0:c0 + 128],
                                    mask3[:, :, 0:128])
                            pipe.append((jt, p))
                        while pipe:
                            jt_, pp_ = pipe.pop(0)
                            emit_av(hp, pav, jt_, pp_, stop=(not pipe))
                        # denominators: free the PSUM accumulators fast,
                        # then normalize via broadcast off the PE path
                        for e in range(2):
                            outu = outup.tile([128, 512], F32, name="outu",
                                              tag=f"outu{e}")
                            nc.vector.tensor_copy(outu[0:64, :], pav[e][0:64, :])
                            dr = rrp.tile([1, 512], F32, name="dr", tag=f"dr{e}")
                            nc.vector.tensor_copy(dr[0:1, :], pav[e][64:65, :])
                            rr = rrp.tile([1, 512], F32, name="rr", tag=f"rr{e}")
                            nc.vector.reciprocal_approx_fast(
                                rr[0:1, :], dr[0:1, :])
                            db = binvp.tile([64, 512], F32, name="db",
                                            tag=f"db{e}")
                            nc.gpsimd.partition_broadcast(db[0:64, :], rr[0:1, :])
                            nc.vector.tensor_mul(
                                outTc[ic][64 * e:64 * (e + 1),
                                          hp * 512: hp * 512 + 512],
                                outu[0:64, :], db[0:64, :])
                        if pending:
                            pending.pop(0)()
                    while pending:
                        pending.pop(0)()
                    pending = make_units(ic, last=(ci == len(CHUNK_ORDER) - 1))
                # tail: the last chunk's output projection
                for fn in pending:
                    fn()

    nc.compile()
    return nc


def _rope_tables():
    inv_freq = 1.0 / (ROPE_BASE ** (np.arange(0, HD, 2, dtype=np.float64) / HD))
    t = np.arange(S, dtype=np.float64)
    freqs = np.outer(t, inv_freq)                      # [S, hd/2]
    emb = np.concatenate([freqs, freqs], axis=-1)      # [S, hd]
    cosT = np.cos(emb).T.astype(np.float32)            # [hd, S]
    sinT = np.sin(emb).T.astype(np.float32)
    cos2 = np.vstack([cosT, cosT])                     # [128, S]
    sin2 = np.vstack([sinT, sinT])
    return np.ascontiguousarray(cos2), np.ascontiguousarray(sin2)


def _rot_matrix():
    r = np.zeros((HD, HD), dtype=np.float32)
    half = HD // 2
    for d in range(half):
        r[d, d + half] = -1.0       # rot(q)[0:32] = -q[32:64]
        r[d + half, d] = 1.0        # rot(q)[32:64] = q[0:32]
    r2 = np.zeros((128, 128), dtype=np.float32)
    r2[0:HD, 0:HD] = r
    r2[HD:128, HD:128] = r
    return np.ascontiguousarray(r2.T)


def _mask_tile():
    # [128, 256]: the same lower-triangle-of-the-diagonal-128-block twice
    # (so a [128, 2, 128] view multiplies both heads of a pair at once)
    jl = np.arange(128)[:, None]
    il = np.arange(128)[None, :]
    tri = (jl <= il).astype(np.float32)
    return np.ascontiguousarray(np.concatenate([tri, tri], axis=1))


def _tile_rows(a):
    """[K*128, C] -> [128, K*C] with row r of tile k at partition r%...:
    a[k*128 + p, :] lands at [p, k*C : (k+1)*C]."""
    kk = a.shape[0] // 128
    return np.ascontiguousarray(
        a.reshape(kk, 128, a.shape[1]).transpose(1, 0, 2).reshape(128, -1))


_prog_cache = {}

# test harness hooks: set TRACE=True before calling kernel() to capture an
# NTFF profile; the BassKernelResults lands in LAST_RESULTS.
TRACE = False
LAST_RESULTS = None


def _f16(a):
    return np.ascontiguousarray(a.astype(np.float16))


def kernel(x, w_qkv, w_out, mask):
    x = np.asarray(x, dtype=np.float32)
    w_qkv = np.asarray(w_qkv, dtype=np.float32)
    w_out = np.asarray(w_out, dtype=np.float32)

    if "nc" not in _prog_cache:
        _prog_cache["nc"] = _build_program()
    nc = _prog_cache["nc"]

    cos2, sin2 = _rope_tables()
    rmatT = _rot_matrix()
    mask2 = _mask_tile()

    in_maps = []
    for c in range(N_CORES):
        b = c // 4
        g = c % 4
        cw = HEADS_PER_CORE * HD   # 256
        wq = w_qkv[:, g * cw:(g + 1) * cw]
        wk = w_qkv[:, D + g * cw: D + (g + 1) * cw]
        wv = w_qkv[:, 2 * D + g * cw: 2 * D + (g + 1) * cw]
        w_c = np.concatenate([wq, wk, wv], axis=1)
        wo_c = w_out[g * cw:(g + 1) * cw, :]
        xT_c = x[b].T
        in_maps.append({
            "xT": _f16(_tile_rows(xT_c)), "w": _f16(_tile_rows(w_c)),
            "wo": _f16(_tile_rows(wo_c)),
            "cosT": _f16(cos2), "sinT": _f16(sin2),
            "rmatT": _f16(rmatT), "mask2": _f16(mask2),
        })

    res = run_bass_kernel_spmd(nc, in_maps, list(range(N_CORES)),
                               trace=TRACE)
    global LAST_RESULTS
    LAST_RESULTS = res
    y = np.zeros((B, S, D), dtype=np.float32)
    for c in range(N_CORES):
        y[c // 4] += res.results[c]["y"].astype(np.float32)
    return y


# revision 22
# speedup vs baseline: 1.2656x; 1.0302x over previous
"""Causal self-attention (B=2, S=2048, D=1024, H=16, hd=64) on 8 TRN2 NeuronCores.

Sharding: batch x head-group. Core c handles batch c//4 and heads
4*(c%4) .. 4*(c%4)+3. Each core computes its 4 heads' attention plus the
partial output projection; the host sums the 4 partial projections per batch.

v2 (vs the 239us baseline):
  - inputs host-pretiled to [128, K*cols] so each tensor loads with one
    contiguous-per-partition DMA; DMAs spread over 4 engine queues so the
    ~1us SWDGE descriptor-gen per dma_start parallelizes (compute starts
    ~3us instead of ~28us).
  - gpsimd ISA library preloaded with a dummy partition_broadcast at t=0
    (the lazy lib load cost ~7us on the first chunk's denominator chain).
  - scores / exp / mask exploit causality inside the diagonal 512-chunk:
    cols < 128*r of a diagonal key-tile are skipped (matmul + exp trimmed,
    probs zero-memset), the 0/1 mask multiply shrinks to the [128,128]
    triangle. Exp for the head pair is one [128, 2, cols] instruction.
  - denominator chain per (hp,e): copy PSUM->SBUF f16 (frees the PSUM
    accumulator ~0.6us after the last AV), reciprocal of the sum row,
    gpsimd partition_broadcast, one f16 multiply. avps needs only 2 banks.
  - output projection of chunk i is emitted inside chunk i+1's score loop
    (PE filler while Act runs exp), chunks processed in order 0,3,2,1 so
    the serial tail is the smallest chunk; y stored f16, one DMA per chunk.
"""

import sys

try:
    import concourse.bass  # noqa: F401
except ImportError:
    sys.path.insert(0, "/opt/trn_rl_repo")

import numpy as np
import concourse.bacc as bacc
import concourse.mybir as mybir
from concourse.tile import TileContext
from concourse.bass_utils import run_bass_kernel_spmd

F32 = mybir.dt.float32
F16 = mybir.dt.float16

B, S, D = 2, 2048, 1024
H, HD = 16, 64
HEADS_PER_CORE = 4
N_CORES = 8
ROPE_BASE = 10000.0
SCALE = HD ** -0.5

KT = D // 128          # 8  contraction tiles for the QKV projection
ST = S // 128          # 16 sequence tiles of 128
NC_CH = S // 512       # 4  sequence chunks of 512
WF = 3 * HEADS_PER_CORE * HD   # 768 projection features per core
VOFF = 2 * HEADS_PER_CORE * HD # 512 column offset of the v block in w

CHUNK_ORDER = [1, 0, 3, 2]


def _build_program():
    nc = bacc.Bacc("TRN2", target_bir_lowering=False, debug=False,
                   num_devices=N_CORES)

    xT = nc.dram_tensor("xT", [128, KT * S], F16, kind="ExternalInput")
    w = nc.dram_tensor("w", [128, KT * WF], F16, kind="ExternalInput")
    wo = nc.dram_tensor("wo", [128, 2 * D], F16, kind="ExternalInput")
    cosT = nc.dram_tensor("cosT", [128, S], F16, kind="ExternalInput")
    sinT = nc.dram_tensor("sinT", [128, S], F16, kind="ExternalInput")
    rmatT = nc.dram_tensor("rmatT", [128, 128], F16, kind="ExternalInput")
    mask2 = nc.dram_tensor("mask2", [128, 256], F16, kind="ExternalInput")
    y = nc.dram_tensor("y", [S, D], F16, kind="ExternalOutput")

    with TileContext(nc) as tc:
        with (
            tc.tile_pool(name="const", bufs=1) as constp,
            tc.tile_pool(name="acts", bufs=1) as actsp,
        ):
            w_sb = constp.tile([128, KT * WF], F16)
            wo_sb = constp.tile([128, 2 * D], F16)
            cos_sb = constp.tile([128, S], F16)
            sin_sb = constp.tile([128, S], F16)
            rmat_sb = constp.tile([128, 128], F16)
            mask_sb = constp.tile([128, 256], F16)
            warm_sb = constp.tile([128, 8], F16)

            # gpsimd ISA library preload: a dummy broadcast at t=0 so the
            # ~7us lazy lib load overlaps the input DMAs.
            nc.vector.memset(warm_sb[0:1, :], 1.0)
            nc.gpsimd.partition_broadcast(warm_sb[64:128, :], warm_sb[0:1, :])

            # input DMAs: only SP/Act (HWDGE) and gpsimd (SWDGE) can issue.
            # x tiles on sync, w tiles + small constants on scalar, bulky
            # late-needed constants on gpsimd (queued behind the lib load).

            # activations produced by the QKV phase, consumed by attention
            qT_sb = actsp.tile([128, 2 * S], F16)   # head pairs 0|1
            kT_sb = actsp.tile([128, 2 * S], F16)
            v_sb = actsp.tile([128, ST * 260], F16) # 16 seq tiles x 4x65
            # per-chunk normalized attention output [d(2 heads), hp*512+q].
            # One tile per chunk so the deferred output projection of chunk
            # i never picks up a (coarse-tracked) dependency on chunk i+1's
            # writes.
            outTc = [actsp.tile([128, 1024], F16, name=f"outT{_c}")
                     for _c in range(NC_CH)]

            # ones columns of the v blocks (col 64 of each 65-block)
            ones_cols = v_sb[:, 0:ST * 260].rearrange(
                "p (b c) -> p b c", c=65)[:, :, 64:65]
            nc.vector.memset(ones_cols, 1.0)

            # ---------------- QKV projection + RoPE ----------------
            with (
                tc.tile_pool(name="xt", bufs=1) as xtp,
                tc.tile_pool(name="qkps", bufs=3, space="PSUM") as qkps,
                tc.tile_pool(name="rotps", bufs=2, space="PSUM") as rotps,
                tc.tile_pool(name="vps", bufs=2, space="PSUM") as vps,
                tc.tile_pool(name="qpre", bufs=3) as qprep,
                tc.tile_pool(name="ropet", bufs=2) as ropetp,
            ):
                xT_sb = xtp.tile([128, KT * S], F16)
                # k ascending so the mt=0 accumulation paces with arrivals;
                # x/w alternate between the two HWDGE queues so tile k needs
                # only ~k transfers on each queue before it lands
                for k in range(KT):
                    qa, qb = (nc.sync, nc.scalar) if k % 2 == 0 else (nc.scalar, nc.sync)
                    qa.dma_start(
                        xT_sb[:, k * S:(k + 1) * S], xT[:, k * S:(k + 1) * S])
                    qb.dma_start(
                        w_sb[:, k * WF:(k + 1) * WF], w[:, k * WF:(k + 1) * WF])
                nc.gpsimd.dma_start(rmat_sb[:], rmatT[:])
                nc.gpsimd.dma_start(cos_sb[:], cosT[:])
                nc.gpsimd.dma_start(sin_sb[:], sinT[:])
                nc.gpsimd.dma_start(wo_sb[:], wo[:])
                nc.gpsimd.dma_start(mask_sb[:], mask2[:])

                # q/k head-pair tiles: mt 0,1 -> q pairs; 2,3 -> k pairs.
                # n-outer accumulation; the RoPE rotation matmul of chunk i
                # is emitted after chunk i+1's accumulation so the PE never
                # waits on the Act-engine qpre copy (keeps the p-state up).
                rope_q = []   # (dest, doff, n, qpre tile)

                def flush_rope():
                    dest, doff, n, qpre = rope_q.pop(0)
                    rot = rotps.tile([128, 512], F32)
                    nc.tensor.matmul(rot[:], rmat_sb[:], qpre[:],
                                     start=True, stop=True)
                    t1 = ropetp.tile([128, 512], F16, tag="t1")
                    t2 = ropetp.tile([128, 512], F16, tag="t2")
                    nc.vector.tensor_mul(
                        t1[:], qpre[:], cos_sb[:, n * 512:(n + 1) * 512])
                    nc.vector.tensor_mul(
                        t2[:], rot[:], sin_sb[:, n * 512:(n + 1) * 512])
                    nc.vector.tensor_add(
                        dest[:, doff + n * 512: doff + (n + 1) * 512],
                        t1[:], t2[:])

                for mt in range(4):
                    dest = qT_sb if mt < 2 else kT_sb
                    doff = (mt % 2) * S
                    for n in range(NC_CH):
                        pt = qkps.tile([128, 512], F32, name="qkpsum",
                                       tag="qkpsum")
                        for k in range(KT):
                            nc.tensor.matmul(
                                pt[:],
                                w_sb[:, k * WF + mt * 128: k * WF + (mt + 1) * 128],
                                xT_sb[:, k * S + n * 512: k * S + (n + 1) * 512],
                                start=(k == 0), stop=(k == KT - 1))
                        qpre = qprep.tile([128, 512], F16)
                        nc.scalar.copy(qpre[:], pt[:])
                        rope_q.append((dest, doff, n, qpre))
                        if len(rope_q) >= 2:
                            flush_rope()
                while rope_q:
                    flush_rope()

                # v in [seq, head-block] layout
                for st in range(ST):
                    pv = vps.tile([128, 256], F32)
                    for k in range(KT):
                        nc.tensor.matmul(
                            pv[:],
                            xT_sb[:, k * S + st * 128: k * S + (st + 1) * 128],
                            w_sb[:, k * WF + VOFF: k * WF + WF],
                            start=(k == 0), stop=(k == KT - 1))
                    vdst = v_sb[:, st * 260:(st + 1) * 260].rearrange(
                        "p (h c) -> p h c", c=65)[:, :, 0:64]
                    nc.vector.tensor_copy(
                        vdst, pv[:].rearrange("p (h c) -> p h c", c=64))

            # ---------------- attention + output projection ----------------
            with (
                tc.tile_pool(name="scps", bufs=2, space="PSUM") as scps,
                tc.tile_pool(name="avps", bufs=1, space="PSUM") as avps,
                tc.tile_pool(name="yps", bufs=2, space="PSUM") as yps,
                tc.tile_pool(name="probs", bufs=5) as probsp,
                tc.tile_pool(name="outu", bufs=2) as outup,
                tc.tile_pool(name="rrp", bufs=2) as rrp,
                tc.tile_pool(name="binv", bufs=2) as binvp,
                tc.tile_pool(name="ysb", bufs=2) as ysbp,
            ):
                mask3 = mask_sb[:, 0:256].rearrange("p (b c) -> p b c", b=2)

                # deferred output-projection units; each unit is one
                # (st, nn) pair: 2 accumulating matmuls + a PSUM->SBUF f16
                # copy into the staging tile; one DMA per seq tile (so the
                # final DMA of the kernel is only 256KB). The last chunk's
                # staging copies go on the Act engine (idle once exps end).
                pending = []   # list of closures for the previous chunk

                def make_units(pc, last=False):
                    ycb = {}

                    def unit(u, pc=pc, ycb=ycb):
                        if u == 0:
                            ycb["t"] = ysbp.tile([128, 4096], F16, name="ycb",
                                                 tag="ycb")
                        sti, nn = u // 2, u % 2
                        py = yps.tile([128, 512], F32, name="py", tag="py")
                        for hp2 in range(2):
                            nc.tensor.matmul(
                                py[:],
                                outTc[pc][:, hp2 * 512 + sti * 128: hp2 * 512 + (sti + 1) * 128],
                                wo_sb[:, hp2 * D + nn * 512: hp2 * D + (nn + 1) * 512],
                                start=(hp2 == 0), stop=(hp2 == 1))
                        ycs = ycb["t"][:, sti * 1024 + nn * 512:
                                       sti * 1024 + (nn + 1) * 512]
                        if last:
                            nc.scalar.copy(ycs, py[:])
                        else:
                            nc.vector.tensor_copy(ycs, py[:])
                        if nn == 1:
                            st = pc * 4 + sti
                            nc.sync.dma_start(
                                y[st * 128:(st + 1) * 128, :],
                                ycb["t"][:, sti * 1024:(sti + 1) * 1024])
                    return [lambda u=u: unit(u) for u in range(8)]

                def emit_av(hp, pav, jt, pp, stop):
                    for e in range(2):
                        h = 2 * hp + e
                        nc.tensor.matmul(
                            pav[e][0:65, :],
                            v_sb[:, jt * 260 + h * 65: jt * 260 + (h + 1) * 65],
                            pp[:, e * 512:(e + 1) * 512],
                            start=(jt == 0), stop=stop)

                def drain(carry):
                    """AV-drain + denominator chain of the previous (ic, hp)
                    stream. Emitted after the next stream's first two score
                    matmuls so the Act engine's exp pipeline never idles at
                    a stream boundary."""
                    ic, hp, pav, pipe = carry
                    while pipe:
                        jt_, pp_ = pipe.pop(0)
                        emit_av(hp, pav, jt_, pp_, stop=(not pipe))
                    for e in range(2):
                        outu = outup.tile([128, 512], F32, name="outu",
                                          tag=f"outu{e}")
                        nc.vector.tensor_copy(outu[0:64, :], pav[e][0:64, :])
                        dr = rrp.tile([1, 512], F32, name="dr", tag=f"dr{e}")
                        nc.vector.tensor_copy(dr[0:1, :], pav[e][64:65, :])
                        rr = rrp.tile([1, 512], F32, name="rr", tag=f"rr{e}")
                        nc.vector.reciprocal_approx_fast(
                            rr[0:1, :], dr[0:1, :])
                        db = binvp.tile([64, 512], F32, name="db",
                                        tag=f"db{e}")
                        nc.gpsimd.partition_broadcast(db[0:64, :], rr[0:1, :])
                        nc.vector.tensor_mul(
                            outTc[ic][64 * e:64 * (e + 1),
                                      hp * 512: hp * 512 + 512],
                            outu[0:64, :], db[0:64, :])

                streams = [(ic, hp) for ic in CHUNK_ORDER for hp in (0, 1)]
                carry = None   # previous stream awaiting AV-drain + denoms
                for si, (ic, hp) in enumerate(streams):
                    jmax = 4 * ic + 4
                    qoffc = ic * 512
                    qoff = hp * S
                    pav = [avps.tile([128, 512], F32, name=f"av{e}",
                                     tag=f"av{e}") for e in range(2)]
                    pipe = []   # (jt, probs tile), AV runs 2 jts behind
                    for jt in range(jmax):
                        r = jt - 4 * ic
                        c0 = 128 * r if r > 0 else 0
                        ps = scps.tile([128, 1024], F32, tag="scps")
                        for e in range(2):
                            psl = slice(64 * e, 64 * (e + 1))
                            nc.tensor.matmul(
                                ps[:, e * 512 + c0:(e + 1) * 512],
                                kT_sb[psl, qoff + jt * 128: qoff + (jt + 1) * 128],
                                qT_sb[psl, qoff + qoffc + c0: qoff + qoffc + 512],
                                start=True, stop=True)
                        if jt == 1 and carry is not None:
                            prev = carry
                            carry = None
                            drain(prev)
                            if prev[1] == 1:   # chunk prev[0] fully done
                                while pending:
                                    pending.pop(0)()
                                pending = make_units(prev[0])
                        elif len(pipe) >= 2:
                            emit_av(hp, pav, *pipe.pop(0), stop=False)
                        # output-projection filler keeps the PE ramped
                        # while Act paces the exp pipeline
                        if jt % 3 == 2 and pending:
                            pending.pop(0)()
                        p = probsp.tile([128, 1024], F16, tag="p")
                        p3 = p[:].rearrange("p (b c) -> p b c", b=2)
                        ps3 = ps[:].rearrange("p (b c) -> p b c", b=2)
                        if c0 > 0:
                            nc.vector.memset(p3[:, :, 0:c0], 0.0)
                        nc.scalar.activation(
                            p3[:, :, c0:512], ps3[:, :, c0:512],
                            mybir.ActivationFunctionType.Exp,
                            scale=SCALE)
                        if r >= 0:
                            nc.vector.tensor_mul(
                                p3[:, :, c0:c0 + 128],
                                p3[:, :, c0:c0 + 128],
                                mask3[:, :, 0:128])
                        pipe.append((jt, p))
                    carry = (ic, hp, pav, pipe)
                # tail: drain the last stream, then its chunk's projection
                drain(carry)
                while pending:
                    pending.pop(0)()
                for fn in make_units(CHUNK_ORDER[-1], last=True):
                    fn()

    nc.compile()
    return nc


def _rope_tables():
    inv_freq = 1.0 / (ROPE_BASE ** (np.arange(0, HD, 2, dtype=np.float64) / HD))
    t = np.arange(S, dtype=np.float64)
    freqs = np.outer(t, inv_freq)                      # [S, hd/2]
    emb = np.concatenate([freqs, freqs], axis=-1)      # [S, hd]
    cosT = np.cos(emb).T.astype(np.float32)            # [hd, S]
    sinT = np.sin(emb).T.astype(np.float32)
    cos2 = np.vstack([cosT, cosT])                     # [128, S]
    sin2 = np.vstack([sinT, sinT])
    return np.ascontiguousarray(cos2), np.ascontiguousarray(sin2)


def _rot_matrix():
    r = np.zeros((HD, HD), dtype=np.float32)
    half = HD // 2
    for d in range(half):
        r[d, d + half] = -1.0       # rot(q)[0:32] = -q[32:64]
        r[d + half, d] = 1.0        # rot(q)[32:64] = q[0:32]
    r2 = np.zeros((128, 128), dtype=np.float32)
    r2[0:HD, 0:HD] = r
    r2[HD:128, HD:128] = r
    return np.ascontiguousarray(r2.T)


def _mask_tile():
    # [128, 256]: the same lower-triangle-of-the-diagonal-128-block twice
    # (so a [128, 2, 128] view multiplies both heads of a pair at once)
    jl = np.arange(128)[:, None]
    il = np.arange(128)[None, :]
    tri = (jl <= il).astype(np.float32)
    return np.ascontiguousarray(np.concatenate([tri, tri], axis=1))


def _tile_rows(a):
    """[K*128, C] -> [128, K*C] with row r of tile k at partition r%...:
    a[k*128 + p, :] lands at [p, k*C : (k+1)*C]."""
    kk = a.shape[0] // 128
    return np.ascontiguousarray(
        a.reshape(kk, 128, a.shape[1]).transpose(1, 0, 2).reshape(128, -1))


_prog_cache = {}

# test harness hooks: set TRACE=True before calling kernel() to capture an
# NTFF profile; the BassKernelResults lands in LAST_RESULTS.
TRACE = False
LAST_RESULTS = None


def _f16(a):
    return np.ascontiguousarray(a.astype(np.float16))


def kernel(x, w_qkv, w_out, mask):
    x = np.asarray(x, dtype=np.float32)
    w_qkv = np.asarray(w_qkv, dtype=np.float32)
    w_out = np.asarray(w_out, dtype=np.float32)

    if "nc" not in _prog_cache:
        _prog_cache["nc"] = _build_program()
    nc = _prog_cache["nc"]

    cos2, sin2 = _rope_tables()
    rmatT = _rot_matrix()
    mask2 = _mask_tile()

    in_maps = []
    for c in range(N_CORES):
        b = c // 4
        g = c % 4
        cw = HEADS_PER_CORE * HD   # 256
        wq = w_qkv[:, g * cw:(g + 1) * cw]
        wk = w_qkv[:, D + g * cw: D + (g + 1) * cw]
        wv = w_qkv[:, 2 * D + g * cw: 2 * D + (g + 1) * cw]
        w_c = np.concatenate([wq, wk, wv], axis=1)
        wo_c = w_out[g * cw:(g + 1) * cw, :]
        xT_c = x[b].T
        in_maps.append({
            "xT": _f16(_tile_rows(xT_c)), "w": _f16(_tile_rows(w_c)),
            "wo": _f16(_tile_rows(wo_c)),
            "cosT": _f16(cos2), "sinT": _f16(sin2),
            "rmatT": _f16(rmatT), "mask2": _f16(mask2),
        })

    res = run_bass_kernel_spmd(nc, in_maps, list(range(N_CORES)),
                               trace=TRACE)
    global LAST_RESULTS
    LAST_RESULTS = res
    y = np.zeros((B, S, D), dtype=np.float32)
    for c in range(N_CORES):
        y[c // 4] += res.results[c]["y"].astype(np.float32)
    return y


# revision 24
# speedup vs baseline: 1.2887x; 1.0182x over previous
"""Causal self-attention (B=2, S=2048, D=1024, H=16, hd=64) on 8 TRN2 NeuronCores.

Sharding: batch x head-group. Core c handles batch c//4 and heads
4*(c%4) .. 4*(c%4)+3. Each core computes its 4 heads' attention plus the
partial output projection; the host sums the 4 partial projections per batch.

v2 (vs the 239us baseline):
  - inputs host-pretiled to [128, K*cols] so each tensor loads with one
    contiguous-per-partition DMA; DMAs spread over 4 engine queues so the
    ~1us SWDGE descriptor-gen per dma_start parallelizes (compute starts
    ~3us instead of ~28us).
  - gpsimd ISA library preloaded with a dummy partition_broadcast at t=0
    (the lazy lib load cost ~7us on the first chunk's denominator chain).
  - scores / exp / mask exploit causality inside the diagonal 512-chunk:
    cols < 128*r of a diagonal key-tile are skipped (matmul + exp trimmed,
    probs zero-memset), the 0/1 mask multiply shrinks to the [128,128]
    triangle. Exp for the head pair is one [128, 2, cols] instruction.
  - denominator chain per (hp,e): copy PSUM->SBUF f16 (frees the PSUM
    accumulator ~0.6us after the last AV), reciprocal of the sum row,
    gpsimd partition_broadcast, one f16 multiply. avps needs only 2 banks.
  - output projection of chunk i is emitted inside chunk i+1's score loop
    (PE filler while Act runs exp), chunks processed in order 0,3,2,1 so
    the serial tail is the smallest chunk; y stored f16, one DMA per chunk.
"""

import sys

try:
    import concourse.bass  # noqa: F401
except ImportError:
    sys.path.insert(0, "/opt/trn_rl_repo")

import numpy as np
import concourse.bacc as bacc
import concourse.mybir as mybir
from concourse.tile import TileContext
from concourse.bass_utils import run_bass_kernel_spmd

F32 = mybir.dt.float32
F16 = mybir.dt.float16

B, S, D = 2, 2048, 1024
H, HD = 16, 64
HEADS_PER_CORE = 4
N_CORES = 8
ROPE_BASE = 10000.0
SCALE = HD ** -0.5

KT = D // 128          # 8  contraction tiles for the QKV projection
ST = S // 128          # 16 sequence tiles of 128
NC_CH = S // 512       # 4  sequence chunks of 512
WF = 3 * HEADS_PER_CORE * HD   # 768 projection features per core
VOFF = 2 * HEADS_PER_CORE * HD # 512 column offset of the v block in w

CHUNK_ORDER = [1, 0, 3, 2]


def _build_program():
    nc = bacc.Bacc("TRN2", target_bir_lowering=False, debug=False,
                   num_devices=N_CORES)

    xT = nc.dram_tensor("xT", [128, KT * S], F16, kind="ExternalInput")
    w = nc.dram_tensor("w", [128, KT * WF], F16, kind="ExternalInput")
    wo = nc.dram_tensor("wo", [128, 2 * D], F16, kind="ExternalInput")
    cosT = nc.dram_tensor("cosT", [128, S], F16, kind="ExternalInput")
    sinT = nc.dram_tensor("sinT", [128, S], F16, kind="ExternalInput")
    rmatT = nc.dram_tensor("rmatT", [128, 128], F16, kind="ExternalInput")
    mask2 = nc.dram_tensor("mask2", [128, 256], F16, kind="ExternalInput")
    y = nc.dram_tensor("y", [S, D], F16, kind="ExternalOutput")

    with TileContext(nc) as tc:
        with (
            tc.tile_pool(name="const", bufs=1) as constp,
            tc.tile_pool(name="acts", bufs=1) as actsp,
        ):
            w_sb = constp.tile([128, KT * WF], F16)
            wo_sb = constp.tile([128, 2 * D], F16)
            cos_sb = constp.tile([128, S], F16)
            sin_sb = constp.tile([128, S], F16)
            rmat_sb = constp.tile([128, 128], F16)
            mask_sb = constp.tile([128, 256], F16)
            warm_sb = constp.tile([128, 8], F16)

            # gpsimd ISA library preload: a dummy broadcast at t=0 so the
            # ~7us lazy lib load overlaps the input DMAs.
            nc.vector.memset(warm_sb[0:1, :], 1.0)
            nc.gpsimd.partition_broadcast(warm_sb[64:128, :], warm_sb[0:1, :])

            # input DMAs: only SP/Act (HWDGE) and gpsimd (SWDGE) can issue.
            # x tiles on sync, w tiles + small constants on scalar, bulky
            # late-needed constants on gpsimd (queued behind the lib load).

            # activations produced by the QKV phase, consumed by attention
            qT_sb = actsp.tile([128, 2 * S], F16)   # head pairs 0|1
            kT_sb = actsp.tile([128, 2 * S], F16)
            v_sb = actsp.tile([128, ST * 260], F16) # 16 seq tiles x 4x65
            # per-chunk normalized attention output [d(2 heads), hp*512+q].
            # One tile per chunk so the deferred output projection of chunk
            # i never picks up a (coarse-tracked) dependency on chunk i+1's
            # writes.
            outTc = [actsp.tile([128, 1024], F16, name=f"outT{_c}")
                     for _c in range(NC_CH)]

            # ones columns of the v blocks (col 64 of each 65-block)
            ones_cols = v_sb[:, 0:ST * 260].rearrange(
                "p (b c) -> p b c", c=65)[:, :, 64:65]
            nc.vector.memset(ones_cols, 1.0)

            # ---------------- QKV projection + RoPE ----------------
            with (
                tc.tile_pool(name="xt", bufs=1) as xtp,
                tc.tile_pool(name="qkps", bufs=4, space="PSUM") as qkps,
                tc.tile_pool(name="rotps", bufs=2, space="PSUM") as rotps,
                tc.tile_pool(name="vps", bufs=2, space="PSUM") as vps,
                tc.tile_pool(name="qpre", bufs=6) as qprep,
                tc.tile_pool(name="ropet", bufs=2) as ropetp,
            ):
                xT_sb = xtp.tile([128, KT * S], F16)
                # k ascending so the mt=0 accumulation paces with arrivals;
                # x/w alternate between the two HWDGE queues so tile k needs
                # only ~k transfers on each queue before it lands
                for k in range(KT):
                    qa, qb = (nc.sync, nc.scalar) if k % 2 == 0 else (nc.scalar, nc.sync)
                    qa.dma_start(
                        xT_sb[:, k * S:(k + 1) * S], xT[:, k * S:(k + 1) * S])
                    qb.dma_start(
                        w_sb[:, k * WF:(k + 1) * WF], w[:, k * WF:(k + 1) * WF])
                nc.gpsimd.dma_start(rmat_sb[:], rmatT[:])
                nc.gpsimd.dma_start(cos_sb[:], cosT[:])
                nc.gpsimd.dma_start(sin_sb[:], sinT[:])
                nc.gpsimd.dma_start(wo_sb[:], wo[:])
                nc.gpsimd.dma_start(mask_sb[:], mask2[:])

                # q/k head-pair tiles: mt 0,1 -> q pairs; 2,3 -> k pairs.
                # n-outer accumulation; the RoPE rotation matmul of chunk i
                # is emitted after chunk i+1's accumulation so the PE never
                # waits on the Act-engine qpre copy (keeps the p-state up).
                rope_q = []   # (dest, doff, n, qpre tile)

                def flush_rope():
                    dest, doff, n, qpre = rope_q.pop(0)
                    rot = rotps.tile([128, 512], F32)
                    nc.tensor.matmul(rot[:], rmat_sb[:], qpre[:],
                                     start=True, stop=True)
                    t1 = ropetp.tile([128, 512], F16, tag="t1")
                    t2 = ropetp.tile([128, 512], F16, tag="t2")
                    nc.vector.tensor_mul(
                        t1[:], qpre[:], cos_sb[:, n * 512:(n + 1) * 512])
                    nc.vector.tensor_mul(
                        t2[:], rot[:], sin_sb[:, n * 512:(n + 1) * 512])
                    nc.vector.tensor_add(
                        dest[:, doff + n * 512: doff + (n + 1) * 512],
                        t1[:], t2[:])

                for mt in range(4):
                    dest = qT_sb if mt < 2 else kT_sb
                    doff = (mt % 2) * S
                    if mt == 0:
                        # k-outer warmup: one matmul per (k, n) as tile k
                        # arrives, so the PE tracks the input DMAs instead
                        # of stalling until the last tile lands
                        pts = [qkps.tile([128, 512], F32, name="qkpsum",
                                         tag="qkpsum") for _ in range(NC_CH)]
                        for k in range(KT):
                            for n in range(NC_CH):
                                nc.tensor.matmul(
                                    pts[n][:],
                                    w_sb[:, k * WF + mt * 128: k * WF + (mt + 1) * 128],
                                    xT_sb[:, k * S + n * 512: k * S + (n + 1) * 512],
                                    start=(k == 0), stop=(k == KT - 1))
                        for n in range(NC_CH):
                            qpre = qprep.tile([128, 512], F16)
                            nc.scalar.copy(qpre[:], pts[n][:])
                            rope_q.append((dest, doff, n, qpre))
                        continue
                    for n in range(NC_CH):
                        pt = qkps.tile([128, 512], F32, name="qkpsum",
                                       tag="qkpsum")
                        for k in range(KT):
                            nc.tensor.matmul(
                                pt[:],
                                w_sb[:, k * WF + mt * 128: k * WF + (mt + 1) * 128],
                                xT_sb[:, k * S + n * 512: k * S + (n + 1) * 512],
                                start=(k == 0), stop=(k == KT - 1))
                        qpre = qprep.tile([128, 512], F16)
                        nc.scalar.copy(qpre[:], pt[:])
                        rope_q.append((dest, doff, n, qpre))
                        while len(rope_q) >= 3:
                            flush_rope()
                while rope_q:
                    flush_rope()

                # v in [seq, head-block] layout
                for st in range(ST):
                    pv = vps.tile([128, 256], F32)
                    for k in range(KT):
                        nc.tensor.matmul(
                            pv[:],
                            xT_sb[:, k * S + st * 128: k * S + (st + 1) * 128],
                            w_sb[:, k * WF + VOFF: k * WF + WF],
                            start=(k == 0), stop=(k == KT - 1))
                    vdst = v_sb[:, st * 260:(st + 1) * 260].rearrange(
                        "p (h c) -> p h c", c=65)[:, :, 0:64]
                    nc.vector.tensor_copy(
                        vdst, pv[:].rearrange("p (h c) -> p h c", c=64))

            # ---------------- attention + output projection ----------------
            with (
                tc.tile_pool(name="scps", bufs=2, space="PSUM") as scps,
                tc.tile_pool(name="avps", bufs=1, space="PSUM") as avps,
                tc.tile_pool(name="yps", bufs=2, space="PSUM") as yps,
                tc.tile_pool(name="probs", bufs=5) as probsp,
                tc.tile_pool(name="outu", bufs=2) as outup,
                tc.tile_pool(name="rrp", bufs=2) as rrp,
                tc.tile_pool(name="binv", bufs=2) as binvp,
                tc.tile_pool(name="ysb", bufs=2) as ysbp,
            ):
                mask3 = mask_sb[:, 0:256].rearrange("p (b c) -> p b c", b=2)

                # deferred output-projection units; each unit is one
                # (st, nn) pair: 2 accumulating matmuls + a PSUM->SBUF f16
                # copy into the staging tile; one DMA per seq tile (so the
                # final DMA of the kernel is only 256KB). The last chunk's
                # staging copies go on the Act engine (idle once exps end).
                pending = []   # list of closures for the previous chunk

                def make_units(pc, last=False):
                    ycb = {}

                    def unit(u, pc=pc, ycb=ycb):
                        if u == 0:
                            ycb["t"] = ysbp.tile([128, 4096], F16, name="ycb",
                                                 tag="ycb")
                        sti, nn = u // 2, u % 2
                        py = yps.tile([128, 512], F32, name="py", tag="py")
                        for hp2 in range(2):
                            nc.tensor.matmul(
                                py[:],
                                outTc[pc][:, hp2 * 512 + sti * 128: hp2 * 512 + (sti + 1) * 128],
                                wo_sb[:, hp2 * D + nn * 512: hp2 * D + (nn + 1) * 512],
                                start=(hp2 == 0), stop=(hp2 == 1))
                        ycs = ycb["t"][:, sti * 1024 + nn * 512:
                                       sti * 1024 + (nn + 1) * 512]
                        if last:
                            nc.scalar.copy(ycs, py[:])
                        else:
                            nc.vector.tensor_copy(ycs, py[:])
                        if nn == 1:
                            st = pc * 4 + sti
                            nc.sync.dma_start(
                                y[st * 128:(st + 1) * 128, :],
                                ycb["t"][:, sti * 1024:(sti + 1) * 1024])
                    return [lambda u=u: unit(u) for u in range(8)]

                def emit_av(hp, pav, jt, pp, stop):
                    for e in range(2):
                        h = 2 * hp + e
                        nc.tensor.matmul(
                            pav[e][0:65, :],
                            v_sb[:, jt * 260 + h * 65: jt * 260 + (h + 1) * 65],
                            pp[:, e * 512:(e + 1) * 512],
                            start=(jt == 0), stop=stop)

                def drain(carry):
                    """AV-drain + denominator chain of the previous (ic, hp)
                    stream. Emitted after the next stream's first two score
                    matmuls so the Act engine's exp pipeline never idles at
                    a stream boundary."""
                    ic, hp, pav, pipe = carry
                    while pipe:
                        jt_, pp_ = pipe.pop(0)
                        emit_av(hp, pav, jt_, pp_, stop=(not pipe))
                    for e in range(2):
                        outu = outup.tile([128, 512], F32, name="outu",
                                          tag=f"outu{e}")
                        nc.vector.tensor_copy(outu[0:64, :], pav[e][0:64, :])
                        dr = rrp.tile([1, 512], F32, name="dr", tag=f"dr{e}")
                        nc.vector.tensor_copy(dr[0:1, :], pav[e][64:65, :])
                        rr = rrp.tile([1, 512], F32, name="rr", tag=f"rr{e}")
                        nc.vector.reciprocal_approx_fast(
                            rr[0:1, :], dr[0:1, :])
                        db = binvp.tile([64, 512], F32, name="db",
                                        tag=f"db{e}")
                        nc.gpsimd.partition_broadcast(db[0:64, :], rr[0:1, :])
                        nc.vector.tensor_mul(
                            outTc[ic][64 * e:64 * (e + 1),
                                      hp * 512: hp * 512 + 512],
                            outu[0:64, :], db[0:64, :])

                streams = [(ic, hp) for ic in CHUNK_ORDER for hp in (0, 1)]
                carry = None   # previous stream awaiting AV-drain + denoms
                for si, (ic, hp) in enumerate(streams):
                    jmax = 4 * ic + 4
                    qoffc = ic * 512
                    qoff = hp * S
                    pav = [avps.tile([128, 512], F32, name=f"av{e}",
                                     tag=f"av{e}") for e in range(2)]
                    pipe = []   # (jt, probs tile), AV runs 2 jts behind
                    for jt in range(jmax):
                        r = jt - 4 * ic
                        c0 = 128 * r if r > 0 else 0
                        ps = scps.tile([128, 1024], F32, tag="scps")
                        for e in range(2):
                            psl = slice(64 * e, 64 * (e + 1))
                            nc.tensor.matmul(
                                ps[:, e * 512 + c0:(e + 1) * 512],
                                kT_sb[psl, qoff + jt * 128: qoff + (jt + 1) * 128],
                                qT_sb[psl, qoff + qoffc + c0: qoff + qoffc + 512],
                                start=True, stop=True)
                        if jt == 1 and carry is not None:
                            prev = carry
                            carry = None
                            drain(prev)
                            if prev[1] == 1:   # chunk prev[0] fully done
                                pending.extend(make_units(prev[0]))
                        elif len(pipe) >= 2:
                            emit_av(hp, pav, *pipe.pop(0), stop=False)
                        # output-projection filler keeps the PE ramped
                        # while Act paces the exp pipeline
                        if jt >= 2 and jt % 2 == 0 and pending:
                            pending.pop(0)()
                        p = probsp.tile([128, 1024], F16, tag="p")
                        p3 = p[:].rearrange("p (b c) -> p b c", b=2)
                        ps3 = ps[:].rearrange("p (b c) -> p b c", b=2)
                        if c0 > 0:
                            nc.vector.memset(p3[:, :, 0:c0], 0.0)
                        nc.scalar.activation(
                            p3[:, :, c0:512], ps3[:, :, c0:512],
                            mybir.ActivationFunctionType.Exp,
                            scale=SCALE)
                        if r >= 0:
                            nc.vector.tensor_mul(
                                p3[:, :, c0:c0 + 128],
                                p3[:, :, c0:c0 + 128],
                                mask3[:, :, 0:128])
                        pipe.append((jt, p))
                    carry = (ic, hp, pav, pipe)
                # tail: drain the last stream, then its chunk's projection
                drain(carry)
                while pending:
                    pending.pop(0)()
                for fn in make_units(CHUNK_ORDER[-1], last=True):
                    fn()

    nc.compile()
    return nc


def _rope_tables():
    inv_freq = 1.0 / (ROPE_BASE ** (np.arange(0, HD, 2, dtype=np.float64) / HD))
    t = np.arange(S, dtype=np.float64)
    freqs = np.outer(t, inv_freq)                      # [S, hd/2]
    emb = np.concatenate([freqs, freqs], axis=-1)      # [S, hd]
    cosT = np.cos(emb).T.astype(np.float32)            # [hd, S]
    sinT = np.sin(emb).T.astype(np.float32)
    cos2 = np.vstack([cosT, cosT])                     # [128, S]
    sin2 = np.vstack([sinT, sinT])
    return np.ascontiguousarray(cos2), np.ascontiguousarray(sin2)


def _rot_matrix():
    r = np.zeros((HD, HD), dtype=np.float32)
    half = HD // 2
    for d in range(half):
        r[d, d + half] = -1.0       # rot(q)[0:32] = -q[32:64]
        r[d + half, d] = 1.0        # rot(q)[32:64] = q[0:32]
    r2 = np.zeros((128, 128), dtype=np.float32)
    r2[0:HD, 0:HD] = r
    r2[HD:128, HD:128] = r
    return np.ascontiguousarray(r2.T)


def _mask_tile():
    # [128, 256]: the same lower-triangle-of-the-diagonal-128-block twice
    # (so a [128, 2, 128] view multiplies both heads of a pair at once)
    jl = np.arange(128)[:, None]
    il = np.arange(128)[None, :]
    tri = (jl <= il).astype(np.float32)
    return np.ascontiguousarray(np.concatenate([tri, tri], axis=1))


def _tile_rows(a):
    """[K*128, C] -> [128, K*C] with row r of tile k at partition r%...:
    a[k*128 + p, :] lands at [p, k*C : (k+1)*C]."""
    kk = a.shape[0] // 128
    return np.ascontiguousarray(
        a.reshape(kk, 128, a.shape[1]).transpose(1, 0, 2).reshape(128, -1))


_prog_cache = {}

# test harness hooks: set TRACE=True before calling kernel() to capture an
# NTFF profile; the BassKernelResults lands in LAST_RESULTS.
TRACE = False
LAST_RESULTS = None


def _f16(a):
    return np.ascontiguousarray(a.astype(np.float16))


def kernel(x, w_qkv, w_out, mask):
    x = np.asarray(x, dtype=np.float32)
    w_qkv = np.asarray(w_qkv, dtype=np.float32)
    w_out = np.asarray(w_out, dtype=np.float32)

    if "nc" not in _prog_cache:
        _prog_cache["nc"] = _build_program()
    nc = _prog_cache["nc"]

    cos2, sin2 = _rope_tables()
    rmatT = _rot_matrix()
    mask2 = _mask_tile()

    in_maps = []
    for c in range(N_CORES):
        b = c // 4
        g = c % 4
        cw = HEADS_PER_CORE * HD   # 256
        wq = w_qkv[:, g * cw:(g + 1) * cw]
        wk = w_qkv[:, D + g * cw: D + (g + 1) * cw]
        wv = w_qkv[:, 2 * D + g * cw: 2 * D + (g + 1) * cw]
        w_c = np.concatenate([wq, wk, wv], axis=1)
        wo_c = w_out[g * cw:(g + 1) * cw, :]
        xT_c = x[b].T
        in_maps.append({
            "xT": _f16(_tile_rows(xT_c)), "w": _f16(_tile_rows(w_c)),
            "wo": _f16(_tile_rows(wo_c)),
            "cosT": _f16(cos2), "sinT": _f16(sin2),
            "rmatT": _f16(rmatT), "mask2": _f16(mask2),
        })

    res = run_bass_kernel_spmd(nc, in_maps, list(range(N_CORES)),
                               trace=TRACE)
    global LAST_RESULTS
    LAST_RESULTS = res
    y = np.zeros((B, S, D), dtype=np.float32)
    for c in range(N_CORES):
        y[c // 4] += res.results[c]["y"].astype(np.float32)
    return y
